# Initial kernel scaffold
#
"""Trainium2 Bass kernel for nn_MaskBBoxPredictor (CoordConv + 3x [conv3x3 +
GroupNorm + ReLU] + per-mask-id segment mean pooling + tiny box/score heads).

Sharding: data-parallel over B=4 images x 2 row-halves = 8 shards on 8 cores.
Each core computes its half-image through the conv stack (with halo rows
recomputed locally), GroupNorm statistics are combined across the 2 cores of
each image with tiny pair-wise AllReduces, and per-segment sums are pooled on
device via PE-transpose + one-hot matmul.  The host gathers the per-core
partial segment sums, combines halves, divides by counts (np.bincount) and
applies the 64->7 / 64->1 heads (negligible FLOPs).

Conv matmuls run as 9 tap-shifted accumulating matmuls per output tile in
float32r (fp32 storage, ~tf32 matmul precision, 4x faster than fp32 on PE).
"""

import sys

for _p in ("/opt/trn_rl_repo", "/root/.axon_site/_ro/trn_rl_repo"):
    if _p not in sys.path:
        sys.path.append(_p)

import numpy as np

import concourse.bass as bass
import concourse.mybir as mybir
import concourse.tile as tile
from concourse import bacc
from concourse.bass_utils import run_bass_kernel_spmd

F32 = mybir.dt.float32
F32R = mybir.dt.float32r
NS = 33
EPS_GN = 1e-5
W = 128
WF = 130  # width + 2 zero border columns
N_CORES = 8


def _r(ap):
    """View an fp32 AP as float32r for PE matmuls."""
    return ap.bitcast(F32R)


def build_program(HH):
    """Build the SPMD Bass program for half-image height HH (=H/2)."""
    LF = HH + 6              # local row frame
    R1L, R1H = 1, HH + 4     # conv1 output rows (inclusive)
    R2L, R2H = 2, HH + 3     # conv2 output rows
    R3L, R3H = 3, HH + 2     # conv3 output rows == owned rows
    OWN_L, OWN_H = 3, HH + 2

    nc = bacc.Bacc("TRN2", target_bir_lowering=False, debug=False,
                   num_devices=N_CORES)

    # ---- kernel I/O -----------------------------------------------------
    xs_d = nc.dram_tensor("xs", [512, LF, WF], F32, kind="ExternalInput")
    gf_d = nc.dram_tensor("gf", [64, LF, WF], F32, kind="ExternalInput")
    w1_d = nc.dram_tensor("w1x", [128, 36, 256], F32, kind="ExternalInput")
    w1g_d = nc.dram_tensor("w1g", [64, 9, 256], F32, kind="ExternalInput")
    w2_d = nc.dram_tensor("w2x", [128, 18, 128], F32, kind="ExternalInput")
    w3_d = nc.dram_tensor("w3x", [128, 9, 64], F32, kind="ExternalInput")
    oh_d = nc.dram_tensor("oneh", [128, HH, NS], F32, kind="ExternalInput")
    g1a_d = nc.dram_tensor("gsel1a", [128, 8], F32, kind="ExternalInput")
    g1b_d = nc.dram_tensor("gsel1b", [128, 8], F32, kind="ExternalInput")
    g2_d = nc.dram_tensor("gsel2", [128, 4], F32, kind="ExternalInput")
    g3_d = nc.dram_tensor("gsel3", [64, 4], F32, kind="ExternalInput")
    gt4_d = nc.dram_tensor("gt4", [4, 128], F32, kind="ExternalInput")
    gt16_d = nc.dram_tensor("gt16", [4, 64], F32, kind="ExternalInput")
    id_d = nc.dram_tensor("ident", [64, 64], F32, kind="ExternalInput")
    pc1_d = nc.dram_tensor("pc1", [256, 3], F32, kind="ExternalInput")
    pc2_d = nc.dram_tensor("pc2", [128, 3], F32, kind="ExternalInput")
    pc3_d = nc.dram_tensor("pc3", [64, 3], F32, kind="ExternalInput")
    rm_d = nc.dram_tensor("rmask", [1, 8], F32, kind="ExternalInput")
    out_d = nc.dram_tensor("pooled", [64, NS], F32, kind="ExternalOutput")

    # collective scratch (Internal, Local)
    ccw_i = nc.dram_tensor("ccw_i", [1, 4], F32, kind="Internal")
    ccw_o = nc.dram_tensor("ccw_o", [1, 4], F32, kind="Internal")
    cc1_i = nc.dram_tensor("cc1_i", [2, 8], F32, kind="Internal")
    cc1_o = nc.dram_tensor("cc1_o", [2, 8], F32, kind="Internal")
    cc2_i = nc.dram_tensor("cc2_i", [2, 4], F32, kind="Internal")
    cc2_o = nc.dram_tensor("cc2_o", [2, 4], F32, kind="Internal")
    cc3_i = nc.dram_tensor("cc3_i", [2, 4], F32, kind="Internal")
    cc3_o = nc.dram_tensor("cc3_o", [2, 4], F32, kind="Internal")
    RG = [[2 * i, 2 * i + 1] for i in range(N_CORES // 2)]

    def allreduce(cin, cout):
        nc.gpsimd.collective_compute(
            "AllReduce", mybir.AluOpType.add, replica_groups=RG,
            ins=[cin[:, :]], outs=[cout[:, :]])

    with tile.TileContext(nc) as tc:
        consts = tc.tile_pool(name="consts", bufs=1).__enter__()
        h1p = tc.tile_pool(name="h1p", bufs=1).__enter__()
        h2p = tc.tile_pool(name="h2p", bufs=1).__enter__()
        h3p = tc.tile_pool(name="h3p", bufs=1).__enter__()
        small = tc.tile_pool(name="small", bufs=1).__enter__()
        sps = tc.tile_pool(name="sps", bufs=2, space="PSUM").__enter__()

        # ---- load constants --------------------------------------------
        w1s = consts.tile([128, 36, 256], F32)
        nc.sync.dma_start(out=w1s, in_=w1_d[:, :, :])
        w1gs = consts.tile([64, 9, 256], F32)
        nc.sync.dma_start(out=w1gs, in_=w1g_d[:, :, :])
        w2s = consts.tile([128, 18, 128], F32)
        nc.sync.dma_start(out=w2s, in_=w2_d[:, :, :])
        w3s = consts.tile([128, 9, 64], F32)
        nc.sync.dma_start(out=w3s, in_=w3_d[:, :, :])
        ohs = consts.tile([128, HH, NS], F32)
        nc.sync.dma_start(out=ohs, in_=oh_d[:, :, :])
        g1as = consts.tile([128, 8], F32)
        nc.sync.dma_start(out=g1as, in_=g1a_d[:, :])
        g1bs = consts.tile([128, 8], F32)
        nc.sync.dma_start(out=g1bs, in_=g1b_d[:, :])
        g2s = consts.tile([128, 4], F32)
        nc.sync.dma_start(out=g2s, in_=g2_d[:, :])
        g3s = consts.tile([64, 4], F32)
        nc.sync.dma_start(out=g3s, in_=g3_d[:, :])
        gt4s = consts.tile([4, 128], F32)
        nc.sync.dma_start(out=gt4s, in_=gt4_d[:, :])
        gt16s = consts.tile([4, 64], F32)
        nc.sync.dma_start(out=gt16s, in_=gt16_d[:, :])
        ids = consts.tile([64, 64], F32)
        nc.sync.dma_start(out=ids, in_=id_d[:, :])
        pc1s = consts.tile([256? if False else 128, 2, 3], F32)  # placeholder
        rmb = consts.tile([128, 8], F32)
        nc.sync.dma_start(
            out=rmb,
            in_=bass.AP(tensor=rm_d, offset=0, ap=[[0, 128], [1, 8]]))

        # per-channel (bias, gamma, beta) per m-chunk
        pc1m = [consts.tile([128, 3], F32, tag=f"pc1_{m}") for m in range(2)]
        for m in range(2):
            nc.sync.dma_start(out=pc1m[m], in_=pc1_d[m * 128:(m + 1) * 128, :])
        pc2s = consts.tile([128, 3], F32)
        nc.sync.dma_start(out=pc2s, in_=pc2_d[:, :])
        pc3s = consts.tile([64, 3], F32)
        nc.sync.dma_start(out=pc3s, in_=pc3_d[:, :])

        # ---- collective warmup (overlaps with conv1) -------------------
        warm = small.tile([1, 4], F32)
        nc.vector.memset(warm, 1.0)
        nc.sync.dma_start(out=ccw_i[:, :], in_=warm)
        allreduce(ccw_i, ccw_o)

        # ---- h buffers --------------------------------------------------
        h1 = [h1p.tile([128, LF, WF], F32, tag=f"h1_{k}") for k in range(2)]
        h2 = h2p.tile([128, LF, WF], F32)
        h3 = h3p.tile([64, HH, W], F32)
        for t in h1 + [h2]:
            nc.vector.memset(t[:, :, 0:1], 0.0)
            nc.vector.memset(t[:, :, WF - 1:WF], 0.0)

        # GN stats scratch
        st1 = [small.tile([128, HH, 6], F32, tag=f"st1_{m}") for m in range(2)]
        st2 = small.tile([128, HH, 6], F32)
        st3 = small.tile([64, HH, 6], F32)

        # =================================================================
        # Phase 1: conv1 (576 -> 256), raw output (bias folded into GN)
        # =================================================================
        with tc.tile_pool(name="xslab", bufs=2) as xp, \
             tc.tile_pool(name="ps1", bufs=4, space="PSUM") as ps1:
            for r0 in range(R1L, R1H + 1, 4):
                slabs = []
                for k in range(4):
                    xt = xp.tile([128, 6, WF], F32, tag=f"xk{k}")
                    nc.sync.dma_start(
                        out=xt, in_=xs_d[k * 128:(k + 1) * 128,
                                         r0 - 1:r0 + 5, :])
                    slabs.append(xt)
                gft = xp.tile([64, 6, WF], F32, tag="gfk")
                nc.sync.dma_start(out=gft, in_=gf_d[:, r0 - 1:r0 + 5, :])

                for m in range(2):
                    ps = ps1.tile([128, 4, W], F32)
                    first = True
                    for t in range(9):
                        dy, dx = t // 3, t % 3
                        for k in range(4):
                            nc.tensor.matmul(
                                ps,
                                _r(w1s[:, t * 4 + k, m * 128:(m + 1) * 128]),
                                _r(slabs[k][:, dy:dy + 4, dx:dx + W]),
                                start=first, stop=False)
                            first = False
                        nc.tensor.matmul(
                            ps,
                            _r(w1gs[:, t, m * 128:(m + 1) * 128]),
                            _r(gft[:, dy:dy + 4, dx:dx + W]),
                            start=False, stop=(t == 8))
                    # copy raw conv out into h1 (cols 1..128)
                    nc.vector.tensor_copy(out=h1[m][:, r0:r0 + 4, 1:W + 1],
                                          in_=ps)
                    # per-row bn_stats over owned rows of this block
                    lo = max(r0, OWN_L)
                    hi = min(r0 + 3, OWN_H)
                    if lo <= hi:
                        nc.vector.bn_stats(
                            out=st1[m][:, lo - OWN_L:hi - OWN_L + 1, :],
                            in_=h1[m][:, lo:hi + 1, 1:W + 1])

        # ---- GN1 sync ---------------------------------------------------
        ssp = sps.tile([2, 8], F32, tag="ssp")
        for m in range(2):
            mv = small.tile([128, 2], F32, tag="mv1")
            nc.vector.bn_aggr(out=mv, in_=st1[m])
            spc = small.tile([128, 2], F32, tag="spc1")
            # col0 = mean + bias ; col1 = var + col0^2
            nc.vector.tensor_add(out=spc[:, 0:1], in0=mv[:, 0:1],
                                 in1=pc1m[m][:, 0:1])
            nc.vector.tensor_mul(out=spc[:, 1:2], in0=spc[:, 0:1],
                                 in1=spc[:, 0:1])
            nc.vector.tensor_add(out=spc[:, 1:2], in0=spc[:, 1:2],
                                 in1=mv[:, 1:2])
            nc.tensor.matmul(ssp, _r(spc), _r(g1as if m == 0 else g1bs),
                             start=(m == 0), stop=(m == 1),
                             skip_group_check=True)
        stg = small.tile([2, 8], F32, tag="stg1")
        nc.vector.tensor_copy(out=stg, in_=ssp)
        nc.sync.dma_start(out=cc1_i[:, :], in_=stg)
        allreduce(cc1_i, cc1_o)
        # load AR result as [group, stat, m]
        s4 = small.tile([4, 2, 2], F32, tag="s4_1")
        nc.sync.dma_start(
            out=s4, in_=bass.AP(tensor=cc1_o, offset=0,
                                ap=[[1, 4], [8, 2], [4, 2]]))
        scale1 = [small.tile([128, 1], F32, tag=f"sc1_{m}") for m in range(2)]
        bias1 = [small.tile([128, 1], F32, tag=f"bi1_{m}") for m in range(2)]

        def gn_post(s4t, nfac, G, gtile, gsz, scs, bis, pcm, nm):
            """Compute per-channel scale/bias tiles from AR'd group sums.
            s4t: [G?, 2, M] tile ([group, stat, mchunk]); gtile: [4, P] sel.
            """
            M = s4t.shape[2]
            mean_t = small.tile([4, M], F32, tag=f"mean_{nm}")
            m2_t = small.tile([4, M], F32, tag=f"m2_{nm}")
            nc.vector.tensor_scalar_mul(out=mean_t, in0=s4t[:, 0, :],
                                        scalar1=float(nfac))
            nc.vector.tensor_scalar_mul(out=m2_t, in0=s4t[:, 1, :],
                                        scalar1=float(nfac))
            var_t = small.tile([4, M], F32, tag=f"var_{nm}")
            nc.vector.tensor_mul(out=var_t, in0=mean_t, in1=mean_t)
            nc.vector.tensor_sub(out=var_t, in0=m2_t, in1=var_t)
            nc.vector.tensor_scalar_add(out=var_t, in0=var_t,
                                        scalar1=float(EPS_GN))
            nc.scalar.activation(out=var_t, in_=var_t,
                                 func=mybir.ActivationFunctionType.Sqrt)
            rstd_t = small.tile([4, M], F32, tag=f"rstd_{nm}")
            nc.vector.reciprocal(out=rstd_t, in_=var_t)
            for m in range(M):
                P = gsz
                psb = sps.tile([128, 1], F32, tag="psb")
                nc.tensor.matmul(psb[:P, :], _r(gtile[:, :P]),
                                 _r(rstd_t[:, m:m + 1]),
                                 start=True, stop=True, skip_group_check=True)
                psm = sps.tile([128, 1], F32, tag="psm")
                nc.tensor.matmul(psm[:P, :], _r(gtile[:, :P]),
                                 _r(mean_t[:, m:m + 1]),
                                 start=True, stop=True, skip_group_check=True)
                # scale = gamma * rstd ; bias = (b - mean) * scale + beta
                nc.vector.tensor_mul(out=scs[m][:P], in0=pcm[m][:P, 1:2],
                                     in1=psb[:P, :])
                tq = small.tile([128, 1], F32, tag=f"tq_{nm}")
                nc.vector.tensor_sub(out=tq[:P], in0=pcm[m][:P, 0:1],
                                     in1=psm[:P, :])
                nc.vector.tensor_mul(out=tq[:P], in0=tq[:P], in1=scs[m][:P])
                nc.vector.tensor_add(out=bis[m][:P], in0=tq[:P],
                                     in1=pcm[m][:P, 2:3])

        gn_post(s4, 1.0 / (2 * 32), 8, gt4s, 128, scale1, bias1, pc1m, "g1")

        # normalize + relu h1 in place (rows R1L..R1H, cols 1..128), then
        # zero out-of-image halo rows
        for m in range(2):
            for r0 in range(R1L, R1H + 1, 4):
                nr = min(4, R1H - r0 + 1)
                nc.scalar.activation(
                    out=h1[m][:, r0:r0 + nr, 1:W + 1],
                    in_=h1[m][:, r0:r0 + nr, 1:W + 1],
                    func=mybir.ActivationFunctionType.Relu,
                    bias=bias1[m], scale=scale1[m])
            for j, r in enumerate([R1L, R1L + 1, R1H - 1, R1H]):
                nc.vector.tensor_scalar_mul(
                    out=h1[m][:, r, 1:W + 1], in0=h1[m][:, r, 1:W + 1],
                    scalar1=rmb[:, j:j + 1])

        # =================================================================
        # Phase 2: conv2 (256 -> 128)
        # =================================================================
        with tc.tile_pool(name="ps2", bufs=4, space="PSUM") as ps2:
            for r0 in range(R2L, R2H + 1, 4):
                nr = min(4, R2H - r0 + 1)
                ps = ps2.tile([128, 4, W], F32)
                first = True
                for t in range(9):
                    dy, dx = t // 3, t % 3
                    for k in range(2):
                        nc.tensor.matmul(
                            ps[:, :nr, :],
                            _r(w2s[:, t * 2 + k, :]),
                            _r(h1[k][:, r0 - 1 + dy:r0 - 1 + dy + nr,
                                     dx:dx + W]),
                            start=first, stop=(t == 8 and k == 1))
                        first = False
                nc.vector.tensor_copy(out=h2[:, r0:r0 + nr, 1:W + 1],
                                      in_=ps[:, :nr, :])
                lo = max(r0, OWN_L)
                hi = min(r0 + nr - 1, OWN_H)
                if lo <= hi:
                    nc.vector.bn_stats(
                        out=st2[:, lo - OWN_L:hi - OWN_L + 1, :],
                        in_=h2[:, lo:hi + 1, 1:W + 1])

        # ---- GN2 sync ---------------------------------------------------
        ssp2 = sps.tile([2, 4], F32, tag="ssp")
        mv2 = small.tile([128, 2], F32, tag="mv2")
        nc.vector.bn_aggr(out=mv2, in_=st2)
        spc2 = small.tile([128, 2], F32, tag="spc2")
        nc.vector.tensor_add(out=spc2[:, 0:1], in0=mv2[:, 0:1],
                             in1=pc2s[:, 0:1])
        nc.vector.tensor_mul(out=spc2[:, 1:2], in0=spc2[:, 0:1],
                             in1=spc2[:, 0:1])
        nc.vector.tensor_add(out=spc2[:, 1:2], in0=spc2[:, 1:2],
                             in1=mv2[:, 1:2])
        nc.tensor.matmul(ssp2, _r(spc2), _r(g2s), start=True, stop=True,
                         skip_group_check=True)
        stg2 = small.tile([2, 4], F32, tag="stg2")
        nc.vector.tensor_copy(out=stg2, in_=ssp2)
        nc.sync.dma_start(out=cc2_i[:, :], in_=stg2)
        allreduce(cc2_i, cc2_o)
        s42 = small.tile([4, 2, 1], F32, tag="s4_2")
        nc.sync.dma_start(
            out=s42, in_=bass.AP(tensor=cc2_o, offset=0,
                                 ap=[[1, 4], [4, 2], [4, 1]]))
        scale2 = [small.tile([128, 1], F32, tag="sc2")]
        bias2 = [small.tile([128, 1], F32, tag="bi2")]
        gn_post(s42, 1.0 / (2 * 32), 4, gt4s, 128, scale2, bias2, [pc2s], "g2")

        for r0 in range(R2L, R2H + 1, 4):
            nr = min(4, R2H - r0 + 1)
            nc.scalar.activation(
                out=h2[:, r0:r0 + nr, 1:W + 1],
                in_=h2[:, r0:r0 + nr, 1:W + 1],
                func=mybir.ActivationFunctionType.Relu,
                bias=bias2[0], scale=scale2[0])
        for j, r in enumerate([R2L, R2H]):
            nc.vector.tensor_scalar_mul(
                out=h2[:, r, 1:W + 1], in0=h2[:, r, 1:W + 1],
                scalar1=rmb[:, 4 + j:5 + j])

        # =================================================================
        # Phase 3: conv3 (128 -> 64)
        # =================================================================
        with tc.tile_pool(name="ps3", bufs=4, space="PSUM") as ps3:
            for r0 in range(R3L, R3H + 1, 4):
                ps = ps3.tile([64, 4, W], F32)
                for t in range(9):
                    dy, dx = t // 3, t % 3
                    nc.tensor.matmul(
                        ps,
                        _r(w3s[:, t, :]),
                        _r(h2[:, r0 - 1 + dy:r0 + 3 + dy, dx:dx + W]),
                        start=(t == 0), stop=(t == 8))
                nc.vector.tensor_copy(out=h3[:, r0 - R3L:r0 - R3L + 4, :],
                                      in_=ps)
                nc.vector.bn_stats(
                    out=st3[:, r0 - R3L:r0 - R3L + 4, :],
                    in_=h3[:, r0 - R3L:r0 - R3L + 4, :])

        # ---- GN3 sync ---------------------------------------------------
        ssp3 = sps.tile([2, 4], F32, tag="ssp")
        mv3 = small.tile([64, 2], F32, tag="mv3")
        nc.vector.bn_aggr(out=mv3, in_=st3)
        spc3 = small.tile([64, 2], F32, tag="spc3")
        nc.vector.tensor_add(out=spc3[:, 0:1], in0=mv3[:, 0:1],
                             in1=pc3s[:, 0:1])
        nc.vector.tensor_mul(out=spc3[:, 1:2], in0=spc3[:, 0:1],
                             in1=spc3[:, 0:1])
        nc.vector.tensor_add(out=spc3[:, 1:2], in0=spc3[:, 1:2],
                             in1=mv3[:, 1:2])
        nc.tensor.matmul(ssp3, _r(spc3), _r(g3s), start=True, stop=True,
                         skip_group_check=True)
        stg3 = small.tile([2, 4], F32, tag="stg3")
        nc.vector.tensor_copy(out=stg3, in_=ssp3)
        nc.sync.dma_start(out=cc3_i[:, :], in_=stg3)
        allreduce(cc3_i, cc3_o)
        s43 = small.tile([4, 2, 1], F32, tag="s4_3")
        nc.sync.dma_start(
            out=s43, in_=bass.AP(tensor=cc3_o, offset=0,
                                 ap=[[1, 4], [4, 2], [4, 1]]))
        scale3 = [small.tile([128, 1], F32, tag="sc3")]
        bias3 = [small.tile([128, 1], F32, tag="bi3")]
        gn_post(s43, 1.0 / (2 * 16), 4, gt16s, 64, scale3, bias3, [pc3s],
                "g3")

        # normalize h3 in place (chunks of 16 rows)
        for r0 in range(0, HH, 16):
            nr = min(16, HH - r0)
            nc.scalar.activation(
                out=h3[:, r0:r0 + nr, :], in_=h3[:, r0:r0 + nr, :],
                func=mybir.ActivationFunctionType.Relu,
                bias=bias3[0][:64], scale=scale3[0][:64])

        # =================================================================
        # Phase 4: segment pooling  pooled[c, s] = sum_px h3n[c, px]*oh[px, s]
        # =================================================================
        with tc.tile_pool(name="pps", bufs=4, space="PSUM") as pps, \
             tc.tile_pool(name="hts", bufs=4) as hts, \
             tc.tile_pool(name="ppool", bufs=1, space="PSUM") as ppool:
            pooled = ppool.tile([64, NS], F32)
            mms = []
            for i in range(HH):
                pT = pps.tile([128, 64], F32)
                nc.tensor.transpose(pT[:, :], h3[:, i, :], ids)
                hT = hts.tile([128, 64], F32)
                nc.vector.tensor_copy(out=hT, in_=pT)
                mms.append((hT, i))
                if len(mms) >= 3:
                    hT2, i2 = mms.pop(0)
                    nc.tensor.matmul(pooled, _r(hT2), _r(ohs[:, i2, :]),
                                     start=(i2 == 0), stop=False,
                                     skip_group_check=True)
            for hT2, i2 in mms:
                nc.tensor.matmul(pooled, _r(hT2), _r(ohs[:, i2, :]),
                                 start=(i2 == 0), stop=(i2 == HH - 1),
                                 skip_group_check=True)
            psb_out = small.tile([64, NS], F32, tag="pout")
            nc.vector.tensor_copy(out=psb_out, in_=pooled)
            nc.sync.dma_start(out=out_d[:, :], in_=psb_out)

    nc.compile()
    return nc


_NC_CACHE = {}


def _get_nc(HH):
    if HH not in _NC_CACHE:
        _NC_CACHE[HH] = build_program(HH)
    return _NC_CACHE[HH]


def _prep_shards(x, masks, w_coord, b_coord, w1, b1, g1, bt1, w2, b2, g2, bt2,
                 w3, b3, g3, bt3):
    B, Cf, H, Wd = x.shape
    assert Wd == W and Cf == 512
    HH = H // 2
    LF = HH + 6

    # grid feats (CoordConv), full image, batch independent
    gy, gx = np.meshgrid(np.arange(H, dtype=np.float32),
                         np.arange(W, dtype=np.float32), indexing="ij")
    wc = w_coord.reshape(64, 2).astype(np.float32)
    gfull = np.maximum(
        wc[:, 0:1, None] * gx[None] + wc[:, 1:2, None] * gy[None]
        + b_coord.reshape(64, 1, 1).astype(np.float32), 0.0)  # [64,H,W]

    # weights, rearranged for tap-shifted matmuls
    w1x = (w1[:, :512].reshape(256, 4, 128, 9).transpose(2, 3, 1, 0)
           .reshape(128, 36, 256).astype(np.float32))
    w1g = w1[:, 512:].reshape(256, 64, 9).transpose(1, 2, 0).astype(np.float32)
    w2x = (w2.reshape(128, 2, 128, 9).transpose(2, 3, 1, 0)
           .reshape(128, 18, 128).astype(np.float32))
    w3x = w3.reshape(64, 128, 9).transpose(1, 2, 0).astype(np.float32)

    p = np.arange(128)
    g8 = np.arange(8)
    gsel1a = ((g8[None, :] < 4) & (p[:, None] // 32 == g8[None, :])
              ).astype(np.float32)
    gsel1b = ((g8[None, :] >= 4) & (p[:, None] // 32 == g8[None, :] - 4)
              ).astype(np.float32)
    g4 = np.arange(4)
    gsel2 = (p[:, None] // 32 == g4[None, :]).astype(np.float32)
    gsel3 = (p[:64, None] // 16 == g4[None, :]).astype(np.float32)
    gt4 = gsel2.T.copy()
    gt16 = gsel3.T.copy()
    ident = np.eye(64, dtype=np.float32)
    pc1 = np.stack([b1, g1, bt1], 1).astype(np.float32)
    pc2 = np.stack([b2, g2, bt2], 1).astype(np.float32)
    pc3 = np.stack([b3, g3, bt3], 1).astype(np.float32)

    in_maps = []
    for b in range(B):
        for half in range(2):
            gofs = half * HH - 3  # local row r -> global row r + gofs
            rlo = max(0, -gofs)
            rhi = min(LF - 1, H - 1 - gofs)
            xs = np.zeros((512, LF, WF), np.float32)
            xs[:, rlo:rhi + 1, 1:W + 1] = x[b][:, rlo + gofs:rhi + 1 + gofs, :]
            gf = np.zeros((64, LF, WF), np.float32)
            gf[:, rlo:rhi + 1, 1:W + 1] = gfull[:, rlo + gofs:rhi + 1 + gofs, :]
            mrows = masks[b, half * HH:(half + 1) * HH, :]  # [HH, W]
            oh = (mrows.T[:, :, None] == np.arange(NS)[None, None, :]
                  ).astype(np.float32)  # [W(part)=128, HH, NS]

            def valid(r):
                g = r + gofs
                return 1.0 if 0 <= g <= H - 1 else 0.0

            rmask = np.array([[valid(1), valid(2), valid(HH + 3),
                               valid(HH + 4), valid(2), valid(HH + 3),
                               0.0, 0.0]], np.float32)
            in_maps.append({
                "xs": xs, "gf": gf, "w1x": w1x, "w1g": w1g, "w2x": w2x,
                "w3x": w3x, "oneh": oh, "gsel1a": gsel1a, "gsel1b": gsel1b,
                "gsel2": gsel2, "gsel3": gsel3, "gt4": gt4, "gt16": gt16,
                "ident": ident, "pc1": pc1, "pc2": pc2, "pc3": pc3,
                "rmask": rmask,
            })
    return in_maps


_LAST_EXEC_NS = None


def kernel(x, masks, w_coord, b_coord, w1, b1, g1, bt1, w2, b2, g2, bt2,
           w3, b3, g3, bt3, w_box, b_box, w_conf, b_conf, _trace=False):
    global _LAST_EXEC_NS
    x = np.asarray(x, np.float32)
    masks = np.asarray(masks)
    B, Cf, H, Wd = x.shape
    HH = H // 2

    in_maps = _prep_shards(x, masks, w_coord, b_coord, w1, b1, g1, bt1,
                           w2, b2, g2, bt2, w3, b3, g3, bt3)
    nc = _get_nc(HH)
    res = run_bass_kernel_spmd(nc, in_maps, core_ids=list(range(N_CORES)),
                               trace=_trace)
    _LAST_EXEC_NS = res.exec_time_ns

    wb = np.asarray(w_box, np.float32).reshape(7, 64)
    wc_ = np.asarray(w_conf, np.float32).reshape(64)
    boxes = np.zeros((B, NS - 1, 7), np.float32)
    scores = np.zeros((B, NS - 1), np.float32)
    for b in range(B):
        sums = (res.results[2 * b]["pooled"]
                + res.results[2 * b + 1]["pooled"])  # [64, NS]
        counts = np.bincount(masks[b].reshape(-1), minlength=NS
                             ).astype(np.float32)
        pooled = sums.T[1:] / np.maximum(counts[1:, None], 1e-4)  # [32, 64]
        boxes[b] = pooled @ wb.T + np.asarray(b_box, np.float32)[None, :]
        scores[b] = pooled @ wc_ + np.asarray(b_conf, np.float32)[0]
    return boxes, scores


# revision 12
# speedup vs baseline: 3.7292x; 3.7292x over previous
"""Trainium2 Bass kernel for nn_MaskBBoxPredictor (CoordConv + 3x [conv3x3 +
GroupNorm + ReLU] + per-mask-id segment mean pooling + tiny box/score heads).

Sharding: data-parallel over B=4 images x 2 row-halves = 8 shards on 8 cores.
Each core computes its half-image through the conv stack (with halo rows
recomputed locally), GroupNorm statistics are combined across the 2 cores of
each image with tiny pair-wise AllReduces, and per-segment sums are pooled on
device via PE-transpose + one-hot matmul.  The host gathers the per-core
partial segment sums, combines halves, divides by counts (np.bincount) and
applies the 64->7 / 64->1 heads (negligible FLOPs).

Conv matmuls run as 9 tap-shifted accumulating matmuls per output tile in
float32r (fp32 storage, ~tf32 matmul precision, 4x faster than fp32 on PE).
"""

import sys

for _p in ("/opt/trn_rl_repo", "/root/.axon_site/_ro/trn_rl_repo"):
    if _p not in sys.path:
        sys.path.append(_p)

import numpy as np

import concourse.bass as bass
import concourse.mybir as mybir
import concourse.tile as tile
from concourse import bacc
from concourse.bass_utils import run_bass_kernel_spmd

F32 = mybir.dt.float32
F32R = mybir.dt.float32r
NS = 33
EPS_GN = 1e-5
W = 128
WF = 130  # width + 2 zero border columns
N_CORES = 8


def _r(ap):
    """View an fp32 AP as float32r for PE matmuls."""
    return ap.bitcast(F32R)


def build_program(HH):
    """Build the SPMD Bass program for half-image height HH (=H/2)."""
    LF = HH + 6              # local row frame
    R1L, R1H = 1, HH + 4     # conv1 output rows (inclusive)
    R2L, R2H = 2, HH + 3     # conv2 output rows
    R3L, R3H = 3, HH + 2     # conv3 output rows == owned rows
    OWN_L, OWN_H = 3, HH + 2

    nc = bacc.Bacc("TRN2", target_bir_lowering=False, debug=False,
                   num_devices=N_CORES)

    # ---- kernel I/O -----------------------------------------------------
    xs_d = nc.dram_tensor("xs", [512, LF, WF], F32R, kind="ExternalInput")
    gf_d = nc.dram_tensor("gf", [64, LF, WF], F32R, kind="ExternalInput")
    w1_d = nc.dram_tensor("w1x", [128, 36, 256], F32R, kind="ExternalInput")
    w1g_d = nc.dram_tensor("w1g", [64, 9, 256], F32R, kind="ExternalInput")
    w2_d = nc.dram_tensor("w2x", [128, 18, 128], F32R, kind="ExternalInput")
    w3_d = nc.dram_tensor("w3x", [128, 9, 64], F32R, kind="ExternalInput")
    oh_d = nc.dram_tensor("oneh", [128, HH, NS], F32, kind="ExternalInput")
    g1a_d = nc.dram_tensor("gsel1a", [128, 8], F32, kind="ExternalInput")
    g1b_d = nc.dram_tensor("gsel1b", [128, 8], F32, kind="ExternalInput")
    g2_d = nc.dram_tensor("gsel2", [128, 4], F32, kind="ExternalInput")
    g3_d = nc.dram_tensor("gsel3", [64, 4], F32, kind="ExternalInput")
    gt4_d = nc.dram_tensor("gt4", [4, 128], F32, kind="ExternalInput")
    gt16_d = nc.dram_tensor("gt16", [4, 64], F32, kind="ExternalInput")
    id_d = nc.dram_tensor("ident", [64, 64], F32R, kind="ExternalInput")
    pc1_d = nc.dram_tensor("pc1", [256, 3], F32, kind="ExternalInput")
    pc2_d = nc.dram_tensor("pc2", [128, 3], F32, kind="ExternalInput")
    pc3_d = nc.dram_tensor("pc3", [64, 3], F32, kind="ExternalInput")
    rm_d = nc.dram_tensor("rmask", [1, 8], F32, kind="ExternalInput")
    zc_d = nc.dram_tensor("zcol", [128, LF], F32R, kind="ExternalInput")
    out_d = nc.dram_tensor("pooled", [64, NS], F32, kind="ExternalOutput")

    # collective scratch (Internal, Local)
    ccw_i = nc.dram_tensor("ccw_i", [1, 4], F32, kind="Internal")
    ccw_o = nc.dram_tensor("ccw_o", [1, 4], F32, kind="Internal")
    cc1_i = nc.dram_tensor("cc1_i", [2, 8], F32, kind="Internal")
    cc1_o = nc.dram_tensor("cc1_o", [2, 8], F32, kind="Internal")
    cc2_i = nc.dram_tensor("cc2_i", [2, 4], F32, kind="Internal")
    cc2_o = nc.dram_tensor("cc2_o", [2, 4], F32, kind="Internal")
    cc3_i = nc.dram_tensor("cc3_i", [2, 4], F32, kind="Internal")
    cc3_o = nc.dram_tensor("cc3_o", [2, 4], F32, kind="Internal")
    RG = [[2 * i, 2 * i + 1] for i in range(N_CORES // 2)]

    def allreduce(cin, cout):
        nc.gpsimd.collective_compute(
            "AllReduce", mybir.AluOpType.add, replica_groups=RG,
            ins=[cin[:, :]], outs=[cout[:, :]])

    from contextlib import ExitStack
    with tile.TileContext(nc) as tc, ExitStack() as es:
        consts = es.enter_context(tc.tile_pool(name="consts", bufs=1))
        h1p = es.enter_context(tc.tile_pool(name="h1p", bufs=1))
        h2p = es.enter_context(tc.tile_pool(name="h2p", bufs=1))
        h3p = es.enter_context(tc.tile_pool(name="h3p", bufs=1))
        small = es.enter_context(tc.tile_pool(name="small", bufs=1))
        sps = es.enter_context(tc.tile_pool(name="sps", bufs=1, space="PSUM"))

        # ---- load constants --------------------------------------------
        w1s = consts.tile([128, 36, 256], F32R)
        nc.sync.dma_start(out=w1s, in_=w1_d[:, :, :])
        w1gs = consts.tile([64, 9, 256], F32R)
        nc.sync.dma_start(out=w1gs, in_=w1g_d[:, :, :])
        w2s = consts.tile([128, 18, 128], F32R)
        nc.sync.dma_start(out=w2s, in_=w2_d[:, :, :])
        w3s = consts.tile([128, 9, 64], F32R)
        nc.sync.dma_start(out=w3s, in_=w3_d[:, :, :])
        ohs = consts.tile([128, HH, NS], F32)
        nc.sync.dma_start(out=ohs, in_=oh_d[:, :, :])
        g1as = consts.tile([128, 8], F32)
        nc.sync.dma_start(out=g1as, in_=g1a_d[:, :])
        g1bs = consts.tile([128, 8], F32)
        nc.sync.dma_start(out=g1bs, in_=g1b_d[:, :])
        g2s = consts.tile([128, 4], F32)
        nc.sync.dma_start(out=g2s, in_=g2_d[:, :])
        g3s = consts.tile([64, 4], F32)
        nc.sync.dma_start(out=g3s, in_=g3_d[:, :])
        gt4s = consts.tile([4, 128], F32)
        nc.sync.dma_start(out=gt4s, in_=gt4_d[:, :])
        gt16s = consts.tile([4, 64], F32)
        nc.sync.dma_start(out=gt16s, in_=gt16_d[:, :])
        ids = consts.tile([64, 64], F32R)
        nc.sync.dma_start(out=ids, in_=id_d[:, :])
        rmb = consts.tile([128, 8], F32)
        nc.sync.dma_start(
            out=rmb,
            in_=bass.AP(tensor=rm_d, offset=0, ap=[[0, 128], [1, 8]]))

        # per-channel (bias, gamma, beta) per m-chunk
        pc1m = [consts.tile([128, 3], F32, tag=f"pc1_{m}", name=f"pc1m{m}") for m in range(2)]
        for m in range(2):
            nc.sync.dma_start(out=pc1m[m], in_=pc1_d[m * 128:(m + 1) * 128, :])
        pc2s = consts.tile([128, 3], F32)
        nc.sync.dma_start(out=pc2s, in_=pc2_d[:, :])
        pc3s = consts.tile([64, 3], F32)
        nc.sync.dma_start(out=pc3s, in_=pc3_d[:, :])

        # ---- collective warmup (overlaps with conv1) -------------------
        warm = small.tile([1, 4], F32)
        nc.vector.memset(warm, 1.0)
        nc.sync.dma_start(out=ccw_i[:, :], in_=warm)
        allreduce(ccw_i, ccw_o)

        # ---- h buffers --------------------------------------------------
        h1 = [h1p.tile([128, LF, WF], F32R, tag=f"h1_{k}", name=f"h1_{k}") for k in range(2)]
        h2 = h2p.tile([128, LF, WF], F32R)
        h3 = h3p.tile([64, HH, W], F32R)
        for t in h1 + [h2]:
            nc.sync.dma_start(out=t[:, :, 0:1], in_=zc_d[:, :])
            nc.sync.dma_start(out=t[:, :, WF - 1:WF], in_=zc_d[:, :])

        # GN stats scratch
        nb1 = len(range(R1L, R1H + 1, 4))
        nb2 = len(range(R2L, R2H + 1, 4))
        nb3 = len(range(R3L, R3H + 1, 4))
        st1 = [small.tile([128, nb1, 2], F32, tag=f"st1_{m}", name=f"st1_{m}")
               for m in range(2)]
        st2 = small.tile([128, nb2, 2], F32)
        st3 = small.tile([64, nb3, 2], F32)
        NT = HH * W  # owned pixels per core

        # =================================================================
        # Phase 1: conv1 (576 -> 256), raw output (bias folded into GN)
        # =================================================================
        with tc.tile_pool(name="xslab", bufs=2) as xp, \
             tc.tile_pool(name="ps1", bufs=4, space="PSUM") as ps1:
            for r0 in range(R1L, R1H + 1, 4):
                slabs = []
                for k in range(4):
                    xt = xp.tile([128, 6, WF], F32R, tag=f"xk{k}")
                    nc.sync.dma_start(
                        out=xt, in_=xs_d[k * 128:(k + 1) * 128,
                                         r0 - 1:r0 + 5, :])
                    slabs.append(xt)
                gft = xp.tile([64, 6, WF], F32R, tag="gfk")
                nc.sync.dma_start(out=gft, in_=gf_d[:, r0 - 1:r0 + 5, :])

                for m in range(2):
                    ps = ps1.tile([128, 4, W], F32)
                    first = True
                    for t in range(9):
                        dy, dx = t // 3, t % 3
                        for k in range(4):
                            nc.tensor.matmul(
                                ps,
                                _r(w1s[:, t * 4 + k, m * 128:(m + 1) * 128]),
                                _r(slabs[k][:, dy:dy + 4, dx:dx + W]),
                                start=first, stop=False)
                            first = False
                        nc.tensor.matmul(
                            ps,
                            _r(w1gs[:, t, m * 128:(m + 1) * 128]),
                            _r(gft[:, dy:dy + 4, dx:dx + W]),
                            start=False, stop=(t == 8))
                    # copy raw conv out into h1 (cols 1..128)
                    nc.vector.tensor_copy(out=h1[m][:, r0:r0 + 4, 1:W + 1],
                                          in_=ps)
                    # per-row bn_stats over owned rows of this block
                    lo = max(r0, OWN_L)
                    hi = min(r0 + 3, OWN_H)
                    if lo <= hi:
                        bi = (r0 - R1L) // 4
                        pso = ps[:, lo - r0:hi - r0 + 1, :]
                        sq = xp.tile([128, 4, W], F32, tag="sqscr",
                                     name="sqscr")
                        nc.scalar.activation(
                            out=sq[:, :hi - lo + 1, :], in_=pso,
                            func=mybir.ActivationFunctionType.Copy,
                            accum_out=st1[m][:, bi, 0:1])
                        nc.scalar.activation(
                            out=sq[:, :hi - lo + 1, :], in_=pso,
                            func=mybir.ActivationFunctionType.Square,
                            accum_out=st1[m][:, bi, 1:2])

        # ---- GN1 sync ---------------------------------------------------
        def sum_to_spc(stt, pcm_t, P, nm):
            """stt [P, nb, 2] block sums -> spc [P, 2] with bias folded:
            col0 = S1 + b*NT ; col1 = S2 + b*(col0 + S1)."""
            nb = stt.shape[1]
            s1 = small.tile([128, 1], F32, tag=f"s1_{nm}", name=f"s1_{nm}")
            s2 = small.tile([128, 1], F32, tag=f"s2_{nm}", name=f"s2_{nm}")
            nc.vector.tensor_reduce(out=s1[:P], in_=stt[:, :, 0:1],
                                    axis=mybir.AxisListType.XY,
                                    op=mybir.AluOpType.add)
            nc.vector.tensor_reduce(out=s2[:P], in_=stt[:, :, 1:2],
                                    axis=mybir.AxisListType.XY,
                                    op=mybir.AluOpType.add)
            spc = small.tile([128, 2], F32, tag=f"spc_{nm}", name=f"spc_{nm}")
            u = small.tile([128, 1], F32, tag=f"u_{nm}", name=f"u_{nm}")
            nc.vector.tensor_scalar_mul(out=u[:P], in0=pcm_t[:P, 0:1],
                                        scalar1=float(NT))
            nc.vector.tensor_add(out=spc[:P, 0:1], in0=s1[:P], in1=u[:P])
            nc.vector.tensor_add(out=u[:P], in0=spc[:P, 0:1], in1=s1[:P])
            nc.vector.tensor_mul(out=u[:P], in0=u[:P], in1=pcm_t[:P, 0:1])
            nc.vector.tensor_add(out=spc[:P, 1:2], in0=s2[:P], in1=u[:P])
            return spc

        ssp = sps.tile([2, 8], F32, tag="ssp")
        for m in range(2):
            spc = sum_to_spc(st1[m], pc1m[m], 128, f"g1{m}")
            nc.tensor.matmul(ssp, spc, (g1as if m == 0 else g1bs),
                             start=(m == 0), stop=(m == 1),
                             skip_group_check=True)
        stg = small.tile([2, 8], F32, tag="stg1")
        nc.vector.tensor_copy(out=stg, in_=ssp)
        nc.sync.dma_start(out=cc1_i[:, :], in_=stg)
        allreduce(cc1_i, cc1_o)
        # load AR result as [group, stat, m]
        s4 = small.tile([4, 2, 2], F32, tag="s4_1")
        nc.sync.dma_start(
            out=s4, in_=bass.AP(tensor=cc1_o, offset=0,
                                ap=[[1, 4], [8, 2], [4, 2]]))
        scale1 = [small.tile([128, 1], F32, tag=f"sc1_{m}", name=f"sc1_{m}") for m in range(2)]
        bias1 = [small.tile([128, 1], F32, tag=f"bi1_{m}", name=f"bi1_{m}") for m in range(2)]

        def gn_post(s4t, nfac, G, gtile, gsz, scs, bis, pcm, nm):
            """Compute per-channel scale/bias tiles from AR'd group sums.
            s4t: [G?, 2, M] tile ([group, stat, mchunk]); gtile: [4, P] sel.
            """
            M = s4t.shape[2]
            mean_t = small.tile([4, M], F32, tag=f"mean_{nm}")
            m2_t = small.tile([4, M], F32, tag=f"m2_{nm}")
            nc.vector.tensor_scalar_mul(out=mean_t, in0=s4t[:, 0, :],
                                        scalar1=float(nfac))
            nc.vector.tensor_scalar_mul(out=m2_t, in0=s4t[:, 1, :],
                                        scalar1=float(nfac))
            var_t = small.tile([4, M], F32, tag=f"var_{nm}")
            nc.vector.tensor_mul(out=var_t, in0=mean_t, in1=mean_t)
            nc.vector.tensor_sub(out=var_t, in0=m2_t, in1=var_t)
            nc.vector.tensor_scalar_add(out=var_t, in0=var_t,
                                        scalar1=float(EPS_GN))
            nc.scalar.activation(out=var_t, in_=var_t,
                                 func=mybir.ActivationFunctionType.Sqrt)
            rstd_t = small.tile([4, M], F32, tag=f"rstd_{nm}")
            nc.vector.reciprocal(out=rstd_t, in_=var_t)
            for m in range(M):
                P = gsz
                psb = sps.tile([128, 1], F32, tag="psb")
                nc.tensor.matmul(psb[:P, :], gtile[:, :P],
                                 rstd_t[:, m:m + 1],
                                 start=True, stop=True, skip_group_check=True)
                psm = sps.tile([128, 1], F32, tag="psm")
                nc.tensor.matmul(psm[:P, :], gtile[:, :P],
                                 mean_t[:, m:m + 1],
                                 start=True, stop=True, skip_group_check=True)
                # scale = gamma * rstd ; bias = (b - mean) * scale + beta
                nc.vector.tensor_mul(out=scs[m][:P], in0=pcm[m][:P, 1:2],
                                     in1=psb[:P, :])
                tq = small.tile([128, 1], F32, tag=f"tq_{nm}")
                nc.vector.tensor_sub(out=tq[:P], in0=pcm[m][:P, 0:1],
                                     in1=psm[:P, :])
                nc.vector.tensor_mul(out=tq[:P], in0=tq[:P], in1=scs[m][:P])
                nc.vector.tensor_add(out=bis[m][:P], in0=tq[:P],
                                     in1=pcm[m][:P, 2:3])

        gn_post(s4, 1.0 / (2 * 32 * NT), 8, gt4s, 128, scale1, bias1, pc1m, "g1")

        # normalize + relu h1 in place (rows R1L..R1H, cols 1..128), then
        # zero out-of-image halo rows
        for m in range(2):
            for r0 in range(R1L, R1H + 1, 4):
                nr = min(4, R1H - r0 + 1)
                nc.scalar.activation(
                    out=h1[m][:, r0:r0 + nr, 1:W + 1],
                    in_=h1[m][:, r0:r0 + nr, 1:W + 1],
                    func=mybir.ActivationFunctionType.Relu,
                    bias=bias1[m], scale=scale1[m])
            for j, r in enumerate([R1L, R1L + 1, R1H - 1, R1H]):
                nc.vector.tensor_scalar_mul(
                    out=h1[m][:, r, 1:W + 1], in0=h1[m][:, r, 1:W + 1],
                    scalar1=rmb[:, j:j + 1])

        # =================================================================
        # Phase 2: conv2 (256 -> 128)
        # =================================================================
        with tc.tile_pool(name="ps2", bufs=4, space="PSUM") as ps2, \
             tc.tile_pool(name="sq2p", bufs=2) as sq2p:
            for r0 in range(R2L, R2H + 1, 4):
                nr = min(4, R2H - r0 + 1)
                ps = ps2.tile([128, 4, W], F32)
                first = True
                for t in range(9):
                    dy, dx = t // 3, t % 3
                    for k in range(2):
                        nc.tensor.matmul(
                            ps[:, :nr, :],
                            _r(w2s[:, t * 2 + k, :]),
                            _r(h1[k][:, r0 - 1 + dy:r0 - 1 + dy + nr,
                                     dx:dx + W]),
                            start=first, stop=(t == 8 and k == 1))
                        first = False
                nc.vector.tensor_copy(out=h2[:, r0:r0 + nr, 1:W + 1],
                                      in_=ps[:, :nr, :])
                lo = max(r0, OWN_L)
                hi = min(r0 + nr - 1, OWN_H)
                if lo <= hi:
                    bi = (r0 - R2L) // 4
                    pso = ps[:, lo - r0:hi - r0 + 1, :]
                    sq = sq2p.tile([128, 4, W], F32, tag="sqscr2",
                                   name="sqscr2")
                    nc.scalar.activation(
                        out=sq[:, :hi - lo + 1, :], in_=pso,
                        func=mybir.ActivationFunctionType.Copy,
                        accum_out=st2[:, bi, 0:1])
                    nc.scalar.activation(
                        out=sq[:, :hi - lo + 1, :], in_=pso,
                        func=mybir.ActivationFunctionType.Square,
                        accum_out=st2[:, bi, 1:2])

        # ---- GN2 sync ---------------------------------------------------
        ssp2 = sps.tile([2, 4], F32, tag="ssp")
        spc2 = sum_to_spc(st2, pc2s, 128, "g2")
        nc.tensor.matmul(ssp2, spc2, g2s, start=True, stop=True,
                         skip_group_check=True)
        stg2 = small.tile([2, 4], F32, tag="stg2")
        nc.vector.tensor_copy(out=stg2, in_=ssp2)
        nc.sync.dma_start(out=cc2_i[:, :], in_=stg2)
        allreduce(cc2_i, cc2_o)
        s42 = small.tile([4, 2, 1], F32, tag="s4_2")
        nc.sync.dma_start(
            out=s42, in_=bass.AP(tensor=cc2_o, offset=0,
                                 ap=[[1, 4], [4, 2], [4, 1]]))
        scale2 = [small.tile([128, 1], F32, tag="sc2", name="sc2")]
        bias2 = [small.tile([128, 1], F32, tag="bi2", name="bi2")]
        gn_post(s42, 1.0 / (2 * 32 * NT), 4, gt4s, 128, scale2, bias2, [pc2s], "g2")

        for r0 in range(R2L, R2H + 1, 4):
            nr = min(4, R2H - r0 + 1)
            nc.scalar.activation(
                out=h2[:, r0:r0 + nr, 1:W + 1],
                in_=h2[:, r0:r0 + nr, 1:W + 1],
                func=mybir.ActivationFunctionType.Relu,
                bias=bias2[0], scale=scale2[0])
        for j, r in enumerate([R2L, R2H]):
            nc.vector.tensor_scalar_mul(
                out=h2[:, r, 1:W + 1], in0=h2[:, r, 1:W + 1],
                scalar1=rmb[:, 4 + j:5 + j])

        # =================================================================
        # Phase 3: conv3 (128 -> 64)
        # =================================================================
        with tc.tile_pool(name="ps3", bufs=4, space="PSUM") as ps3, \
             tc.tile_pool(name="sq3p", bufs=2) as sq3p:
            for r0 in range(R3L, R3H + 1, 4):
                ps = ps3.tile([64, 4, W], F32)
                for t in range(9):
                    dy, dx = t // 3, t % 3
                    nc.tensor.matmul(
                        ps,
                        _r(w3s[:, t, :]),
                        _r(h2[:, r0 - 1 + dy:r0 + 3 + dy, dx:dx + W]),
                        start=(t == 0), stop=(t == 8))
                nc.vector.tensor_copy(out=h3[:, r0 - R3L:r0 - R3L + 4, :],
                                      in_=ps)
                bi = (r0 - R3L) // 4
                sq = sq3p.tile([64, 4, W], F32, tag="sqscr3", name="sqscr3")
                nc.scalar.activation(
                    out=sq, in_=ps,
                    func=mybir.ActivationFunctionType.Copy,
                    accum_out=st3[:, bi, 0:1])
                nc.scalar.activation(
                    out=sq, in_=ps,
                    func=mybir.ActivationFunctionType.Square,
                    accum_out=st3[:, bi, 1:2])

        # ---- GN3 sync ---------------------------------------------------
        ssp3 = sps.tile([2, 4], F32, tag="ssp")
        spc3_full = sum_to_spc(st3, pc3s, 64, "g3")
        spc3 = spc3_full[:64]
        nc.tensor.matmul(ssp3, spc3, g3s, start=True, stop=True,
                         skip_group_check=True)
        stg3 = small.tile([2, 4], F32, tag="stg3")
        nc.vector.tensor_copy(out=stg3, in_=ssp3)
        nc.sync.dma_start(out=cc3_i[:, :], in_=stg3)
        allreduce(cc3_i, cc3_o)
        s43 = small.tile([4, 2, 1], F32, tag="s4_3")
        nc.sync.dma_start(
            out=s43, in_=bass.AP(tensor=cc3_o, offset=0,
                                 ap=[[1, 4], [4, 2], [4, 1]]))
        scale3 = [small.tile([128, 1], F32, tag="sc3", name="sc3")]
        bias3 = [small.tile([128, 1], F32, tag="bi3", name="bi3")]
        gn_post(s43, 1.0 / (2 * 16 * NT), 4, gt16s, 64, scale3, bias3,
                [pc3s], "g3")

        # normalize h3 in place (chunks of 16 rows)
        for r0 in range(0, HH, 16):
            nr = min(16, HH - r0)
            nc.scalar.activation(
                out=h3[:, r0:r0 + nr, :], in_=h3[:, r0:r0 + nr, :],
                func=mybir.ActivationFunctionType.Relu,
                bias=bias3[0][:64], scale=scale3[0][:64])

        # =================================================================
        # Phase 4: segment pooling  pooled[c, s] = sum_px h3n[c, px]*oh[px, s]
        # =================================================================
        with tc.tile_pool(name="pps", bufs=4, space="PSUM") as pps, \
             tc.tile_pool(name="hts", bufs=4) as hts, \
             tc.tile_pool(name="ppool", bufs=1, space="PSUM") as ppool:
            pooled = ppool.tile([64, NS], F32)
            mms = []
            for i in range(HH):
                pT = pps.tile([128, 64], F32R)
                nc.tensor.transpose(pT[:, :], h3[:, i, :], ids)
                hT = hts.tile([128, 64], F32)
                nc.vector.tensor_copy(out=hT, in_=pT)
                mms.append((hT, i))
                if len(mms) >= 3:
                    hT2, i2 = mms.pop(0)
                    nc.tensor.matmul(pooled, hT2, ohs[:, i2, :],
                                     start=(i2 == 0), stop=False,
                                     skip_group_check=True)
            for hT2, i2 in mms:
                nc.tensor.matmul(pooled, hT2, ohs[:, i2, :],
                                 start=(i2 == 0), stop=(i2 == HH - 1),
                                 skip_group_check=True)
            psb_out = small.tile([64, NS], F32, tag="pout")
            nc.vector.tensor_copy(out=psb_out, in_=pooled)
            nc.sync.dma_start(out=out_d[:, :], in_=psb_out)

    nc.compile()
    return nc


_NC_CACHE = {}


def _get_nc(HH):
    if HH not in _NC_CACHE:
        _NC_CACHE[HH] = build_program(HH)
    return _NC_CACHE[HH]


def _prep_shards(x, masks, w_coord, b_coord, w1, b1, g1, bt1, w2, b2, g2, bt2,
                 w3, b3, g3, bt3):
    B, Cf, H, Wd = x.shape
    assert Wd == W and Cf == 512
    HH = H // 2
    LF = HH + 6

    # grid feats (CoordConv), full image, batch independent
    gy, gx = np.meshgrid(np.arange(H, dtype=np.float32),
                         np.arange(W, dtype=np.float32), indexing="ij")
    wc = w_coord.reshape(64, 2).astype(np.float32)
    gfull = np.maximum(
        wc[:, 0:1, None] * gx[None] + wc[:, 1:2, None] * gy[None]
        + b_coord.reshape(64, 1, 1).astype(np.float32), 0.0)  # [64,H,W]

    # weights, rearranged for tap-shifted matmuls
    w1x = (w1[:, :512].reshape(256, 4, 128, 9).transpose(2, 3, 1, 0)
           .reshape(128, 36, 256).astype(np.float32))
    w1g = w1[:, 512:].reshape(256, 64, 9).transpose(1, 2, 0).astype(np.float32)
    w2x = (w2.reshape(128, 2, 128, 9).transpose(2, 3, 1, 0)
           .reshape(128, 18, 128).astype(np.float32))
    w3x = w3.reshape(64, 128, 9).transpose(1, 2, 0).astype(np.float32)

    p = np.arange(128)
    g8 = np.arange(8)
    gsel1a = ((g8[None, :] < 4) & (p[:, None] // 32 == g8[None, :])
              ).astype(np.float32)
    gsel1b = ((g8[None, :] >= 4) & (p[:, None] // 32 == g8[None, :] - 4)
              ).astype(np.float32)
    g4 = np.arange(4)
    gsel2 = (p[:, None] // 32 == g4[None, :]).astype(np.float32)
    gsel3 = (p[:64, None] // 16 == g4[None, :]).astype(np.float32)
    gt4 = gsel2.T.copy()
    gt16 = gsel3.T.copy()
    ident = np.eye(64, dtype=np.float32)
    pc1 = np.stack([b1, g1, bt1], 1).astype(np.float32)
    pc2 = np.stack([b2, g2, bt2], 1).astype(np.float32)
    pc3 = np.stack([b3, g3, bt3], 1).astype(np.float32)

    in_maps = []
    for b in range(B):
        for half in range(2):
            gofs = half * HH - 3  # local row r -> global row r + gofs
            rlo = max(0, -gofs)
            rhi = min(LF - 1, H - 1 - gofs)
            xs = np.zeros((512, LF, WF), np.float32)
            xs[:, rlo:rhi + 1, 1:W + 1] = x[b][:, rlo + gofs:rhi + 1 + gofs, :]
            gf = np.zeros((64, LF, WF), np.float32)
            gf[:, rlo:rhi + 1, 1:W + 1] = gfull[:, rlo + gofs:rhi + 1 + gofs, :]
            mrows = masks[b, half * HH:(half + 1) * HH, :]  # [HH, W]
            oh = (mrows.T[:, :, None] == np.arange(NS)[None, None, :]
                  ).astype(np.float32)  # [W(part)=128, HH, NS]

            def valid(r):
                g = r + gofs
                return 1.0 if 0 <= g <= H - 1 else 0.0

            rmask = np.array([[valid(1), valid(2), valid(HH + 3),
                               valid(HH + 4), valid(2), valid(HH + 3),
                               0.0, 0.0]], np.float32)
            in_maps.append({
                "xs": xs, "gf": gf, "w1x": w1x, "w1g": w1g, "w2x": w2x,
                "w3x": w3x, "oneh": oh, "gsel1a": gsel1a, "gsel1b": gsel1b,
                "gsel2": gsel2, "gsel3": gsel3, "gt4": gt4, "gt16": gt16,
                "ident": ident, "pc1": pc1, "pc2": pc2, "pc3": pc3,
                "rmask": rmask, "zcol": np.zeros((128, LF), np.float32),
            })
    return in_maps


_LAST_EXEC_NS = None


def kernel(x, masks, w_coord, b_coord, w1, b1, g1, bt1, w2, b2, g2, bt2,
           w3, b3, g3, bt3, w_box, b_box, w_conf, b_conf, _trace=False):
    global _LAST_EXEC_NS
    x = np.asarray(x, np.float32)
    masks = np.asarray(masks)
    B, Cf, H, Wd = x.shape
    HH = H // 2

    in_maps = _prep_shards(x, masks, w_coord, b_coord, w1, b1, g1, bt1,
                           w2, b2, g2, bt2, w3, b3, g3, bt3)
    nc = _get_nc(HH)
    res = run_bass_kernel_spmd(nc, in_maps, core_ids=list(range(N_CORES)),
                               trace=_trace)
    _LAST_EXEC_NS = res.exec_time_ns

    wb = np.asarray(w_box, np.float32).reshape(7, 64)
    wc_ = np.asarray(w_conf, np.float32).reshape(64)
    boxes = np.zeros((B, NS - 1, 7), np.float32)
    scores = np.zeros((B, NS - 1), np.float32)
    for b in range(B):
        sums = (res.results[2 * b]["pooled"]
                + res.results[2 * b + 1]["pooled"])  # [64, NS]
        counts = np.bincount(masks[b].reshape(-1), minlength=NS
                             ).astype(np.float32)
        pooled = sums.T[1:] / np.maximum(counts[1:, None], 1e-4)  # [32, 64]
        boxes[b] = pooled @ wb.T + np.asarray(b_box, np.float32)[None, :]
        scores[b] = pooled @ wc_ + np.asarray(b_conf, np.float32)[0]
    return boxes, scores
